# revision 26
# baseline (speedup 1.0000x reference)
# BinarizeLinear on 8 Trainium2 NeuronCores.
#
# reference: out = binarize(x) @ binarize(weight).T + bias
#   x      [16384, 2048] f32
#   weight [2048, 2048]  f32
#   bias   [2048]        f32
#   out    [16384, 2048] f32
#
# Strategy (data-parallel over rows of x, weight/bias replicated):
#   - Each of the 8 cores gets a 2048-row shard of x.
#   - Host uploads x-shard and weight TRANSPOSED (K on the leading axis) and
#     SIGN-PACKED 2-per-byte: byte = s_j0<<7 | s_j1<<6 | 0x1C, where s=1 iff
#     elem<=0 (reference maps 0 -> -1) and j indexes the two DoubleRow
#     k-planes of a strip.  Signs are all binarize consumes, so this is
#     lossless and halves input HBM traffic (8MB -> 4MB per core), which
#     makes the startup ramp PE-bound instead of DMA-bound.
#   - The packed strip DMAs straight into the j=0 plane of the expanded
#     [128, 2, NF] fp8 tile.  Two tensor_scalar passes reconstruct exact
#     +-1.0 fp8 operands:
#       plane1 = (pk << 1) & 0xB8B8   (s_j1<<7 | 0x38 per byte)
#       plane0 = (pk & 0x8080) | 0x3838  (in place)
#     The 0x1C filler makes the plane1 shift land exactly on 0x38 (+1.0)
#     with a single 2-op pass; bit0 cross-byte spill is masked by 0xB8B8.
#   - out.T[n, m] = sum_k wbT[k, n] * xbT[k, m] accumulates in PSUM with
#     DoubleRow fp8 matmuls (2 MACs/cell/cycle, contraction 256 per MM).
#   - ScalarE evacuates PSUM with a fused per-partition bias add
#     (activation Identity, bias = bias[n] column) directly to f16 output
#     tiles (values are +-2048-range integers + bias: f16 keeps rel err
#     ~2e-4, and halves output HBM traffic 16MB -> 8MB).
#   - Output DMAs ride the GpSimd queue (idle engine) so they never delay
#     PSUM evacuation on ScalarE or the input streams on Sync/ScalarE
#     queues.
#   - PE warm-up: 8 dummy DoubleRow matmuls on a garbage tile flip the HAM
#     clock gate (needs ~3.4us of sustained PE activity at the boot-time
#     half clock) before real data lands.  They accumulate into the PSUM
#     pair that the first real group touches LAST (ps1_1), so no real
#     matmul ever waits on the warm-up chain.
#   - Host transposes each core's out.T shard back, casts f32, and stacks.

import sys

import numpy as np

try:
    import concourse  # noqa: F401
except ImportError:
    sys.path.insert(0, "/opt/trn_rl_repo")

from contextlib import ExitStack

import ml_dtypes

import concourse.bass as bass
import concourse.mybir as mybir
import concourse.tile as tile
from concourse import bacc
from concourse.bass_utils import run_bass_kernel_spmd

NCORES = 8
K = 2048          # contraction dim (in_features)
NF = 2048         # out features
MTOT = 16384      # rows of x
MS = MTOT // NCORES  # rows per core
P = 128           # partitions
MC = 512          # moving free-dim chunk (one PSUM bank of f32)
KT2 = K // (2 * P)   # 8 double-k-tiles (DoubleRow contracts 256/MM)
NT = NF // P      # 16 n-tiles
MT = MS // MC     # 4 m-chunks

F32 = mybir.dt.float32
F16 = mybir.dt.float16
FP8 = mybir.dt.float8e4
U8 = mybir.dt.uint8
U16 = mybir.dt.uint16


def build_nc(debug=False):
    nc = bacc.Bacc(
        "TRN2", target_bir_lowering=False, debug=debug, num_devices=NCORES
    )
    # Strip 0 arrives PRE-EXPANDED (fp8 +-1, 512KB): its load is a single
    # DMA with no vector work, so the first real matmul depends only on two
    # DMA semaphores (w0 on the sync queue head, x0 on the gpsimd queue
    # head) and fires ~2us earlier than a packed strip 0 could.  Strips
    # 1..7 arrive sign-packed ([t, p, c] u8, one contiguous 256KB DMA per
    # strip, 2KB per partition line) since their expansion hides behind
    # strip-0's consumption.
    x0f = nc.dram_tensor("x0f", [P, 2, MS], FP8, kind="ExternalInput").ap()
    w0f = nc.dram_tensor("w0f", [P, 2, NF], FP8, kind="ExternalInput").ap()
    xT = nc.dram_tensor("xT", [KT2 - 1, P, MS], U8, kind="ExternalInput").ap()
    wT = nc.dram_tensor("wT", [KT2 - 1, P, NF], U8, kind="ExternalInput").ap()
    # bias arrives pre-tiled [128, 16] from the host (column t holds
    # bias[t*128:(t+1)*128]) so the DMA is one contiguous 8KB transfer.
    bias = nc.dram_tensor("bias", [P, NT], F32, kind="ExternalInput").ap()
    outT = nc.dram_tensor("outT", [NF, MS], F16, kind="ExternalOutput").ap()

    NG = 2  # n-tiles per group; NG*MT psum banks live at once

    with tile.TileContext(nc) as tc:
        with ExitStack() as ctx:
            const = ctx.enter_context(tc.tile_pool(name="const", bufs=1))
            res = ctx.enter_context(tc.tile_pool(name="res", bufs=1))
            psum = ctx.enter_context(
                tc.tile_pool(name="ps", bufs=1, space=bass.MemorySpace.PSUM)
            )
            outp = ctx.enter_context(tc.tile_pool(name="out", bufs=3))

            # PE warm-up (see header).  Memset is Vector's first op (Vector
            # is idle until the first strip lands ~4us later), so the
            # warm-up starts right at the PE preamble end (~6.5us), flips
            # the HAM clock at ~10us, and bridges gap-free into the first
            # real matmul at strip0-ready (~11.5us): the clock gate needs
            # ~3.4us of CONTINUOUS PE activity, so a gap between warm-up
            # and first real MM would push the flip ~4us later.
            warm = const.tile([P, 2, MC], FP8, name="warm")
            nc.vector.memset(warm[:], 0.0)
            warm_ps = psum.tile([P, 2 * MC], F32, tag="ps1_1", name="warm_ps")

            def load_bin(dram, name, t, dma_eng, alu_eng, nsplit=1):
                # Packed strip [128, 2048] u8 DMAs into the j=0 plane of the
                # [128, 2, NF] fp8 operand tile; two tensor_scalar passes
                # expand to exact +-1.0 (plane1 first: it reads raw packed
                # bytes that the plane0 pass overwrites in place).  nsplit>1
                # loads/expands the strip in column slabs: Tile dependencies
                # are byte-range-based, so the first matmuls fire as soon as
                # their slab is ready -- used on strip 0 to get real work to
                # the PE right at the HAM clock flip instead of waiting for
                # the whole strip.
                tl = res.tile([P, 2, NF], FP8, tag=f"{name}{t}")
                cs = NF // nsplit
                for s in range(nsplit):
                    sl = slice(s * cs, (s + 1) * cs)
                    dma_eng.dma_start(
                        out=tl[:, 0, sl].bitcast(U8), in_=dram[t][:, sl]
                    )
                    alu_eng.tensor_scalar(
                        tl[:, 1, sl].bitcast(U16),
                        tl[:, 0, sl].bitcast(U16),
                        1,
                        0xB8B8,
                        mybir.AluOpType.logical_shift_left,
                        mybir.AluOpType.bitwise_and,
                    )
                    alu_eng.tensor_scalar(
                        tl[:, 0, sl].bitcast(U16),
                        tl[:, 0, sl].bitcast(U16),
                        0x8080,
                        0x3838,
                        mybir.AluOpType.bitwise_and,
                        mybir.AluOpType.bitwise_or,
                    )
                return tl

            # w strips on the sync queue, x strips on the scalar queue ->
            # the two HWDGE queues stream in parallel.  Both expand on
            # Vector (the Pool engine can't run tensor_scalar): 4 passes
            # of ~0.4us per strip-pair still beats the ~1.4us DMA cadence.
            # Pre-expanded strip 0: head-of-queue DMAs on sync (w0) and
            # gpsimd (x0) so both 512KB transfers run in parallel and land
            # ~11.3us, with no expansion pass and no coupling to the vector
            # instruction ordinal (Tile cross-engine waits are counting
            # semaphores on the producer's ordinal, so keeping strip 0 off
            # the vector queue keeps its dependency exact).
            w0 = res.tile([P, 2, NF], FP8, tag="w0")
            x0 = res.tile([P, 2, NF], FP8, tag="x0")
            nc.sync.dma_start(out=w0[:], in_=w0f[:])
            nc.gpsimd.dma_start(out=x0[:], in_=x0f[:])

            # bias next on sync (8KB, negligible); needed at ~20us.
            bias_t = const.tile([P, NT], F32)
            nc.sync.dma_start(out=bias_t[:], in_=bias[:])

            # PE warm-up bridges the PE-preamble end (~6.5us) to strip-0
            # readiness (~11.5us) with CONTINUOUS activity: the HAM
            # clock-flip window resets on any PE gap, and a reset costs
            # ~2us of half-clock real matmuls.  8 warm-ups at the 427ns
            # half clock end ~11.9us, right at both the flip and strip-0
            # readiness.
            NWARM = 8
            for wi in range(NWARM):
                nc.tensor.matmul(
                    warm_ps[:, :MC],
                    warm[:, :, :P],
                    warm[:],
                    start=(wi == 0),
                    stop=(wi == NWARM - 1),
                    perf_mode=mybir.MatmulPerfMode.DoubleRow,
                )

            # Packed strips 1..7: x rides the scalar queue, w the sync queue
            # (behind w0+bias).  Issue x before w per t so the vector queue
            # order is x1,w1,x2,w2,...: each pass's data lands before the
            # pass ahead of it blocks the queue, and every strip t is
            # expanded before the PE (consuming 1.73us/strip from ~11.9us)
            # needs it.
            wb = [w0]
            xb = [x0]
            for t in range(1, KT2):
                xb.append(load_bin(xT, f"x{t}_", t - 1, nc.scalar, nc.vector))
                wb.append(load_bin(wT, f"w{t}_", t - 1, nc.sync, nc.vector))

            def w_slice(t, n):
                return wb[t][:, :, n * P : (n + 1) * P]

            def x_slice(t, mc):
                return xb[t][:, :, mc * MC : (mc + 1) * MC]

            # PSUM is organized as pair-tiles [128, 2*MC] spanning two banks:
            # each matmul still writes within a single bank (one MC slice),
            # but evacuation reads a whole pair in one ACTIVATE -- halving
            # the evacuation ops and the cross-engine semaphore edges (the
            # kernel epilogue's final drain pays ~tens of ns per allocated
            # semaphore, so edge count shows up on the wall clock).
            NPAIR = MT // 2
            NGRP = NT // NG

            def mm(ps_pair, g, i, mc, t):
                nc.tensor.matmul(
                    ps_pair[:, (mc % 2) * MC : (mc % 2 + 1) * MC],
                    w_slice(t, g * NG + i),
                    x_slice(t, mc),
                    start=(t == 0),
                    stop=(t == KT2 - 1),
                    perf_mode=mybir.MatmulPerfMode.DoubleRow,
                )

            for g in range(NGRP):
                pss = [
                    [
                        psum.tile(
                            [P, 2 * MC], F32, tag=f"ps{i}_{pr}", name=f"ps_{g}_{i}_{pr}"
                        )
                        for pr in range(NPAIR)
                    ]
                    for i in range(NG)
                ]
                ots = [
                    outp.tile([P, MS], F16, tag=f"o{i}", name=f"o_{g}_{i}")
                    for i in range(NG)
                ]

                def evacuate(i, pr, dma):
                    n = g * NG + i
                    if dma == "chunk":
                        # Final pair: evacuate + DMA in two 512-col chunks on
                        # the Sync queue (idle once inputs land, and not
                        # backed up behind the earlier output transfers like
                        # GpSimd's), so the tail is one small chunk's
                        # epilogue.
                        for c in range(2):
                            sl = slice(
                                (2 * pr + c) * MC, (2 * pr + c + 1) * MC
                            )
                            nc.scalar.activation(
                                ots[i][:, sl],
                                pss[i][pr][:, c * MC : (c + 1) * MC],
                                mybir.ActivationFunctionType.Identity,
                                bias=bias_t[:, n : n + 1],
                            )
                            nc.sync.dma_start(
                                out=outT[n * P : (n + 1) * P, sl],
                                in_=ots[i][:, sl],
                            )
                        return
                    nc.scalar.activation(
                        ots[i][:, pr * 2 * MC : (pr + 1) * 2 * MC],
                        pss[i][pr][:],
                        mybir.ActivationFunctionType.Identity,
                        bias=bias_t[:, n : n + 1],
                    )
                    # Output DMAs ride the (otherwise idle) GpSimd queue.
                    # dma=None batches the whole n-tile into one transfer;
                    # the last group DMAs per-pair for tail overlap.
                    if dma == "pair":
                        nc.sync.dma_start(
                            out=outT[
                                n * P : (n + 1) * P, pr * 2 * MC : (pr + 1) * 2 * MC
                            ],
                            in_=ots[i][:, pr * 2 * MC : (pr + 1) * 2 * MC],
                        )
                    elif dma == "tile":
                        nc.gpsimd.dma_start(
                            out=outT[n * P : (n + 1) * P, :], in_=ots[i][:]
                        )

                if g < NGRP - 1:
                    # k-tile outer: consume input strips as they stream in.
                    for t in range(KT2):
                        for i in range(NG):
                            for mc in range(MT):
                                mm(pss[i][mc // 2], g, i, mc, t)
                    for i in range(NG):
                        for pr in range(NPAIR):
                            evacuate(i, pr, "tile" if pr == NPAIR - 1 else None)
                else:
                    # Last group: pair-major so evacuation and output DMA of
                    # pair p overlap the matmuls of pair p+1 (shrinks the
                    # kernel tail to one pair's epilogue).
                    for i in range(NG):
                        for pr in range(NPAIR):
                            for mc in (2 * pr, 2 * pr + 1):
                                for t in range(KT2):
                                    mm(pss[i][pr], g, i, mc, t)
                            last = i == NG - 1 and pr == NPAIR - 1
                            evacuate(i, pr, "chunk" if last else "pair")

            # Trailing dummy matmuls: the HAM clock gate halves the clock
            # ~3.3us after the PE goes idle, which would put the final output
            # drain and the fixed ~250-semaphore epilogue wipe at half speed.
            # ~16 throwaway DoubleRow MMs (3.5us) keep the PE "busy" through
            # the drain window so the teardown runs at full clock.  They
            # reuse the ps0_0 bank (evacuated early in the last group) and
            # are never read.
            tail_ps = psum.tile([P, 2 * MC], F32, tag="ps0_0", name="tail_ps")
            NDUMMY = 16
            for wi in range(NDUMMY):
                nc.tensor.matmul(
                    tail_ps[:, :MC],
                    warm[:, :, :P],
                    warm[:],
                    start=(wi == 0),
                    stop=(wi == NDUMMY - 1),
                    perf_mode=mybir.MatmulPerfMode.DoubleRow,
                )

    nc.compile()
    return nc


_NC = None


def _get_nc():
    global _NC
    if _NC is None:
        _NC = build_nc()
    return _NC


def _pack_T(a):
    # Transposed sign encode.  Strip 0 (k rows 0..255) goes out PRE-EXPANDED
    # as fp8 +-1 bytes (0x38 / 0xB8); strips 1..7 sign-packed 2-per-byte:
    # byte = s_j0<<7 | s_j1<<6 | 0x1C with s = (elem <= 0)  (reference
    # binarize maps 0 -> -1).  j indexes the two DoubleRow k-planes:
    # element [t, j, p, c] = a.T[(2t+j)*128 + p, c].
    at = a.T
    kk, cols = at.shape
    s = (at <= 0).reshape(kk // (2 * P), 2, P, cols)
    pk = (
        (s[1:, 0].astype(np.uint8) << 7)
        | (s[1:, 1].astype(np.uint8) << 6)
        | np.uint8(0x1C)
    )
    f0 = np.where(s[0], np.uint8(0xB8), np.uint8(0x38)).transpose(1, 0, 2)
    f0 = np.ascontiguousarray(f0).view(ml_dtypes.float8_e4m3fn)
    return f0, np.ascontiguousarray(pk)


def make_in_maps(x, weight, bias):
    x = np.asarray(x, dtype=np.float32)
    weight = np.asarray(weight, dtype=np.float32)
    bias = np.asarray(bias, dtype=np.float32)
    w0f, wTb = _pack_T(weight)
    bias_tiled = np.ascontiguousarray(bias.reshape(NT, P).T)
    in_maps = []
    for i in range(NCORES):
        x0f, xTb = _pack_T(x[i * MS : (i + 1) * MS, :])
        in_maps.append(
            {
                "x0f": x0f,
                "w0f": w0f,
                "xT": xTb,
                "wT": wTb,
                "bias": bias_tiled,
            }
        )
    return in_maps


def assemble_out(results):
    out = np.empty((MTOT, NF), dtype=np.float32)
    for i in range(NCORES):
        out[i * MS : (i + 1) * MS, :] = results[i]["outT"].T.astype(np.float32)
    return out


def run(x, weight, bias, trace=False, **kwargs):
    nc = _get_nc()
    in_maps = make_in_maps(x, weight, bias)
    res = run_bass_kernel_spmd(
        nc, in_maps, list(range(NCORES)), trace=trace, **kwargs
    )
    return assemble_out(res.results), res


def kernel(x, weight, bias):
    out, _ = run(x, weight, bias)
    return out


# revision 27
# speedup vs baseline: 1.0477x; 1.0477x over previous
# BinarizeLinear on 8 Trainium2 NeuronCores.
#
# reference: out = binarize(x) @ binarize(weight).T + bias
#   x      [16384, 2048] f32
#   weight [2048, 2048]  f32
#   bias   [2048]        f32
#   out    [16384, 2048] f32
#
# Strategy (data-parallel over rows of x, weight/bias replicated):
#   - Each of the 8 cores gets a 2048-row shard of x.
#   - Host uploads x-shard and weight TRANSPOSED (K on the leading axis) and
#     SIGN-PACKED 2-per-byte: byte = s_j0<<7 | s_j1<<6 | 0x1C, where s=1 iff
#     elem<=0 (reference maps 0 -> -1) and j indexes the two DoubleRow
#     k-planes of a strip.  Signs are all binarize consumes, so this is
#     lossless and halves input HBM traffic (8MB -> 4MB per core), which
#     makes the startup ramp PE-bound instead of DMA-bound.
#   - The packed strip DMAs straight into the j=0 plane of the expanded
#     [128, 2, NF] fp8 tile.  Two tensor_scalar passes reconstruct exact
#     +-1.0 fp8 operands:
#       plane1 = (pk << 1) & 0xB8B8   (s_j1<<7 | 0x38 per byte)
#       plane0 = (pk & 0x8080) | 0x3838  (in place)
#     The 0x1C filler makes the plane1 shift land exactly on 0x38 (+1.0)
#     with a single 2-op pass; bit0 cross-byte spill is masked by 0xB8B8.
#   - out.T[n, m] = sum_k wbT[k, n] * xbT[k, m] accumulates in PSUM with
#     DoubleRow fp8 matmuls (2 MACs/cell/cycle, contraction 256 per MM).
#   - ScalarE evacuates PSUM with a fused per-partition bias add
#     (activation Identity, bias = bias[n] column) directly to f16 output
#     tiles (values are +-2048-range integers + bias: f16 keeps rel err
#     ~2e-4, and halves output HBM traffic 16MB -> 8MB).
#   - Output DMAs ride the GpSimd queue (idle engine) so they never delay
#     PSUM evacuation on ScalarE or the input streams on Sync/ScalarE
#     queues.
#   - PE warm-up: 8 dummy DoubleRow matmuls on a garbage tile flip the HAM
#     clock gate (needs ~3.4us of sustained PE activity at the boot-time
#     half clock) before real data lands.  They accumulate into the PSUM
#     pair that the first real group touches LAST (ps1_1), so no real
#     matmul ever waits on the warm-up chain.
#   - Host transposes each core's out.T shard back, casts f32, and stacks.

import sys

import numpy as np

try:
    import concourse  # noqa: F401
except ImportError:
    sys.path.insert(0, "/opt/trn_rl_repo")

from contextlib import ExitStack

import ml_dtypes

import concourse.bass as bass
import concourse.mybir as mybir
import concourse.tile as tile
from concourse import bacc
from concourse.bass_utils import run_bass_kernel_spmd

NCORES = 8
K = 2048          # contraction dim (in_features)
NF = 2048         # out features
MTOT = 16384      # rows of x
MS = MTOT // NCORES  # rows per core
P = 128           # partitions
MC = 512          # moving free-dim chunk (one PSUM bank of f32)
KT2 = K // (2 * P)   # 8 double-k-tiles (DoubleRow contracts 256/MM)
NT = NF // P      # 16 n-tiles
MT = MS // MC     # 4 m-chunks

F32 = mybir.dt.float32
F16 = mybir.dt.float16
FP8 = mybir.dt.float8e4
U8 = mybir.dt.uint8
U16 = mybir.dt.uint16


def build_nc(debug=False):
    nc = bacc.Bacc(
        "TRN2", target_bir_lowering=False, debug=debug, num_devices=NCORES
    )
    # Strip 0 arrives PRE-EXPANDED (fp8 +-1, 512KB): its load is a single
    # DMA with no vector work, so the first real matmul depends only on two
    # DMA semaphores (w0 on the sync queue head, x0 on the gpsimd queue
    # head) and fires ~2us earlier than a packed strip 0 could.  Strips
    # 1..7 arrive sign-packed ([t, p, c] u8, one contiguous 256KB DMA per
    # strip, 2KB per partition line) since their expansion hides behind
    # strip-0's consumption.
    x0f = nc.dram_tensor("x0f", [P, 2, MS], FP8, kind="ExternalInput").ap()
    w0f = nc.dram_tensor("w0f", [P, 2, NF], FP8, kind="ExternalInput").ap()
    xT = nc.dram_tensor("xT", [KT2 - 1, P, MS], U8, kind="ExternalInput").ap()
    wT = nc.dram_tensor("wT", [KT2 - 1, P, NF], U8, kind="ExternalInput").ap()
    # bias arrives pre-tiled [128, 16] from the host (column t holds
    # bias[t*128:(t+1)*128]) so the DMA is one contiguous 8KB transfer.
    bias = nc.dram_tensor("bias", [P, NT], F32, kind="ExternalInput").ap()
    outT = nc.dram_tensor("outT", [NF, MS], F16, kind="ExternalOutput").ap()

    NG = 2  # n-tiles per group; NG*MT psum banks live at once

    with tile.TileContext(nc) as tc:
        with ExitStack() as ctx:
            const = ctx.enter_context(tc.tile_pool(name="const", bufs=1))
            res = ctx.enter_context(tc.tile_pool(name="res", bufs=1))
            psum = ctx.enter_context(
                tc.tile_pool(name="ps", bufs=1, space=bass.MemorySpace.PSUM)
            )
            outp = ctx.enter_context(tc.tile_pool(name="out", bufs=3))

            # PE warm-up (see header).  Memset is Vector's first op (Vector
            # is idle until the first strip lands ~4us later), so the
            # warm-up starts right at the PE preamble end (~6.5us), flips
            # the HAM clock at ~10us, and bridges gap-free into the first
            # real matmul at strip0-ready (~11.5us): the clock gate needs
            # ~3.4us of CONTINUOUS PE activity, so a gap between warm-up
            # and first real MM would push the flip ~4us later.
            warm = const.tile([P, 2, MC], FP8, name="warm")
            nc.vector.memset(warm[:], 0.0)
            warm_ps = psum.tile([P, 2 * MC], F32, tag="ps1_1", name="warm_ps")

            def load_bin(dram, name, t, dma_eng, alu_eng, nsplit=1):
                # Packed strip [128, 2048] u8 DMAs into the j=0 plane of the
                # [128, 2, NF] fp8 operand tile; two tensor_scalar passes
                # expand to exact +-1.0 (plane1 first: it reads raw packed
                # bytes that the plane0 pass overwrites in place).  nsplit>1
                # loads/expands the strip in column slabs: Tile dependencies
                # are byte-range-based, so the first matmuls fire as soon as
                # their slab is ready -- used on strip 0 to get real work to
                # the PE right at the HAM clock flip instead of waiting for
                # the whole strip.
                tl = res.tile([P, 2, NF], FP8, tag=f"{name}{t}")
                cs = NF // nsplit
                for s in range(nsplit):
                    sl = slice(s * cs, (s + 1) * cs)
                    dma_eng.dma_start(
                        out=tl[:, 0, sl].bitcast(U8), in_=dram[t][:, sl]
                    )
                    alu_eng.tensor_scalar(
                        tl[:, 1, sl].bitcast(U16),
                        tl[:, 0, sl].bitcast(U16),
                        1,
                        0xB8B8,
                        mybir.AluOpType.logical_shift_left,
                        mybir.AluOpType.bitwise_and,
                    )
                    alu_eng.tensor_scalar(
                        tl[:, 0, sl].bitcast(U16),
                        tl[:, 0, sl].bitcast(U16),
                        0x8080,
                        0x3838,
                        mybir.AluOpType.bitwise_and,
                        mybir.AluOpType.bitwise_or,
                    )
                return tl

            # w strips on the sync queue, x strips on the scalar queue ->
            # the two HWDGE queues stream in parallel.  Both expand on
            # Vector (the Pool engine can't run tensor_scalar): 4 passes
            # of ~0.4us per strip-pair still beats the ~1.4us DMA cadence.
            # Pre-expanded strip 0: head-of-queue DMAs on sync (w0) and
            # scalar (x0) so both 512KB transfers run in parallel and land
            # ~11.3us, with no expansion pass and no coupling to the vector
            # instruction ordinal (Tile cross-engine waits are counting
            # semaphores on the producer's ordinal, so keeping strip 0 off
            # the vector queue keeps its dependency exact).  NOT the gpsimd
            # queue: it measured ~2x slower for this transfer and the 3us
            # slip re-halved the HAM clock for 10us.
            w0 = res.tile([P, 2, NF], FP8, tag="w0")
            x0 = res.tile([P, 2, NF], FP8, tag="x0")
            nc.sync.dma_start(out=w0[:], in_=w0f[:])
            nc.scalar.dma_start(out=x0[:], in_=x0f[:])

            # bias next on sync (8KB, negligible); needed at ~20us.
            bias_t = const.tile([P, NT], F32)
            nc.sync.dma_start(out=bias_t[:], in_=bias[:])

            # PE warm-up bridges the PE-preamble end (~6.5us) to strip-0
            # readiness (~11.5us) with CONTINUOUS activity: the HAM
            # clock-flip window resets on any PE gap, and a reset costs
            # ~2us of half-clock real matmuls.  8 warm-ups at the 427ns
            # half clock end ~11.9us, right at both the flip and strip-0
            # readiness.
            NWARM = 8
            for wi in range(NWARM):
                nc.tensor.matmul(
                    warm_ps[:, :MC],
                    warm[:, :, :P],
                    warm[:],
                    start=(wi == 0),
                    stop=(wi == NWARM - 1),
                    perf_mode=mybir.MatmulPerfMode.DoubleRow,
                )

            # Packed strips 1..7: x rides the scalar queue, w the sync queue
            # (behind w0+bias).  Issue x before w per t so the vector queue
            # order is x1,w1,x2,w2,...: each pass's data lands before the
            # pass ahead of it blocks the queue, and every strip t is
            # expanded before the PE (consuming 1.73us/strip from ~11.9us)
            # needs it.
            wb = [w0]
            xb = [x0]
            for t in range(1, KT2):
                xb.append(load_bin(xT, f"x{t}_", t - 1, nc.scalar, nc.vector))
                wb.append(load_bin(wT, f"w{t}_", t - 1, nc.sync, nc.vector))

            def w_slice(t, n):
                return wb[t][:, :, n * P : (n + 1) * P]

            def x_slice(t, mc):
                return xb[t][:, :, mc * MC : (mc + 1) * MC]

            # PSUM is organized as pair-tiles [128, 2*MC] spanning two banks:
            # each matmul still writes within a single bank (one MC slice),
            # but evacuation reads a whole pair in one ACTIVATE -- halving
            # the evacuation ops and the cross-engine semaphore edges (the
            # kernel epilogue's final drain pays ~tens of ns per allocated
            # semaphore, so edge count shows up on the wall clock).
            NPAIR = MT // 2
            NGRP = NT // NG

            def mm(ps_pair, g, i, mc, t):
                nc.tensor.matmul(
                    ps_pair[:, (mc % 2) * MC : (mc % 2 + 1) * MC],
                    w_slice(t, g * NG + i),
                    x_slice(t, mc),
                    start=(t == 0),
                    stop=(t == KT2 - 1),
                    perf_mode=mybir.MatmulPerfMode.DoubleRow,
                )

            for g in range(NGRP):
                pss = [
                    [
                        psum.tile(
                            [P, 2 * MC], F32, tag=f"ps{i}_{pr}", name=f"ps_{g}_{i}_{pr}"
                        )
                        for pr in range(NPAIR)
                    ]
                    for i in range(NG)
                ]
                ots = [
                    outp.tile([P, MS], F16, tag=f"o{i}", name=f"o_{g}_{i}")
                    for i in range(NG)
                ]

                def evacuate(i, pr, dma):
                    n = g * NG + i
                    if dma == "chunk":
                        # Final pair: evacuate + DMA in two 512-col chunks on
                        # the Sync queue (idle once inputs land, and not
                        # backed up behind the earlier output transfers like
                        # GpSimd's), so the tail is one small chunk's
                        # epilogue.
                        for c in range(2):
                            sl = slice(
                                (2 * pr + c) * MC, (2 * pr + c + 1) * MC
                            )
                            nc.scalar.activation(
                                ots[i][:, sl],
                                pss[i][pr][:, c * MC : (c + 1) * MC],
                                mybir.ActivationFunctionType.Identity,
                                bias=bias_t[:, n : n + 1],
                            )
                            nc.sync.dma_start(
                                out=outT[n * P : (n + 1) * P, sl],
                                in_=ots[i][:, sl],
                            )
                        return
                    nc.scalar.activation(
                        ots[i][:, pr * 2 * MC : (pr + 1) * 2 * MC],
                        pss[i][pr][:],
                        mybir.ActivationFunctionType.Identity,
                        bias=bias_t[:, n : n + 1],
                    )
                    # Output DMAs ride the (otherwise idle) GpSimd queue.
                    # dma=None batches the whole n-tile into one transfer;
                    # the last group DMAs per-pair for tail overlap.
                    if dma == "pair":
                        nc.sync.dma_start(
                            out=outT[
                                n * P : (n + 1) * P, pr * 2 * MC : (pr + 1) * 2 * MC
                            ],
                            in_=ots[i][:, pr * 2 * MC : (pr + 1) * 2 * MC],
                        )
                    elif dma == "tile":
                        nc.gpsimd.dma_start(
                            out=outT[n * P : (n + 1) * P, :], in_=ots[i][:]
                        )

                if g < NGRP - 1:
                    # k-tile outer: consume input strips as they stream in.
                    for t in range(KT2):
                        for i in range(NG):
                            for mc in range(MT):
                                mm(pss[i][mc // 2], g, i, mc, t)
                    for i in range(NG):
                        for pr in range(NPAIR):
                            evacuate(i, pr, "tile" if pr == NPAIR - 1 else None)
                else:
                    # Last group: pair-major so evacuation and output DMA of
                    # pair p overlap the matmuls of pair p+1 (shrinks the
                    # kernel tail to one pair's epilogue).
                    for i in range(NG):
                        for pr in range(NPAIR):
                            for mc in (2 * pr, 2 * pr + 1):
                                for t in range(KT2):
                                    mm(pss[i][pr], g, i, mc, t)
                            last = i == NG - 1 and pr == NPAIR - 1
                            evacuate(i, pr, "chunk" if last else "pair")

            # Trailing dummy matmuls: the HAM clock gate halves the clock
            # ~3.3us after the PE goes idle, which would put the final output
            # drain and the fixed ~250-semaphore epilogue wipe at half speed.
            # ~16 throwaway DoubleRow MMs (3.5us) keep the PE "busy" through
            # the drain window so the teardown runs at full clock.  They
            # reuse the ps0_0 bank (evacuated early in the last group) and
            # are never read.
            tail_ps = psum.tile([P, 2 * MC], F32, tag="ps0_0", name="tail_ps")
            NDUMMY = 16
            for wi in range(NDUMMY):
                nc.tensor.matmul(
                    tail_ps[:, :MC],
                    warm[:, :, :P],
                    warm[:],
                    start=(wi == 0),
                    stop=(wi == NDUMMY - 1),
                    perf_mode=mybir.MatmulPerfMode.DoubleRow,
                )

    nc.compile()
    return nc


_NC = None


def _get_nc():
    global _NC
    if _NC is None:
        _NC = build_nc()
    return _NC


def _pack_T(a):
    # Transposed sign encode.  Strip 0 (k rows 0..255) goes out PRE-EXPANDED
    # as fp8 +-1 bytes (0x38 / 0xB8); strips 1..7 sign-packed 2-per-byte:
    # byte = s_j0<<7 | s_j1<<6 | 0x1C with s = (elem <= 0)  (reference
    # binarize maps 0 -> -1).  j indexes the two DoubleRow k-planes:
    # element [t, j, p, c] = a.T[(2t+j)*128 + p, c].
    at = a.T
    kk, cols = at.shape
    s = (at <= 0).reshape(kk // (2 * P), 2, P, cols)
    pk = (
        (s[1:, 0].astype(np.uint8) << 7)
        | (s[1:, 1].astype(np.uint8) << 6)
        | np.uint8(0x1C)
    )
    f0 = np.where(s[0], np.uint8(0xB8), np.uint8(0x38)).transpose(1, 0, 2)
    f0 = np.ascontiguousarray(f0).view(ml_dtypes.float8_e4m3fn)
    return f0, np.ascontiguousarray(pk)


def make_in_maps(x, weight, bias):
    x = np.asarray(x, dtype=np.float32)
    weight = np.asarray(weight, dtype=np.float32)
    bias = np.asarray(bias, dtype=np.float32)
    w0f, wTb = _pack_T(weight)
    bias_tiled = np.ascontiguousarray(bias.reshape(NT, P).T)
    in_maps = []
    for i in range(NCORES):
        x0f, xTb = _pack_T(x[i * MS : (i + 1) * MS, :])
        in_maps.append(
            {
                "x0f": x0f,
                "w0f": w0f,
                "xT": xTb,
                "wT": wTb,
                "bias": bias_tiled,
            }
        )
    return in_maps


def assemble_out(results):
    out = np.empty((MTOT, NF), dtype=np.float32)
    for i in range(NCORES):
        out[i * MS : (i + 1) * MS, :] = results[i]["outT"].T.astype(np.float32)
    return out


def run(x, weight, bias, trace=False, **kwargs):
    nc = _get_nc()
    in_maps = make_in_maps(x, weight, bias)
    res = run_bass_kernel_spmd(
        nc, in_maps, list(range(NCORES)), trace=trace, **kwargs
    )
    return assemble_out(res.results), res


def kernel(x, weight, bias):
    out, _ = run(x, weight, bias)
    return out


# revision 29
# speedup vs baseline: 1.0559x; 1.0078x over previous
# BinarizeLinear on 8 Trainium2 NeuronCores.
#
# reference: out = binarize(x) @ binarize(weight).T + bias
#   x      [16384, 2048] f32
#   weight [2048, 2048]  f32
#   bias   [2048]        f32
#   out    [16384, 2048] f32
#
# Strategy (data-parallel over rows of x, weight/bias replicated):
#   - Each of the 8 cores gets a 2048-row shard of x.
#   - Host uploads x-shard and weight TRANSPOSED (K on the leading axis) and
#     SIGN-PACKED 2-per-byte: byte = s_j0<<7 | s_j1<<6 | 0x1C, where s=1 iff
#     elem<=0 (reference maps 0 -> -1) and j indexes the two DoubleRow
#     k-planes of a strip.  Signs are all binarize consumes, so this is
#     lossless and halves input HBM traffic (8MB -> 4MB per core), which
#     makes the startup ramp PE-bound instead of DMA-bound.
#   - The packed strip DMAs straight into the j=0 plane of the expanded
#     [128, 2, NF] fp8 tile.  Two tensor_scalar passes reconstruct exact
#     +-1.0 fp8 operands:
#       plane1 = (pk << 1) & 0xB8B8   (s_j1<<7 | 0x38 per byte)
#       plane0 = (pk & 0x8080) | 0x3838  (in place)
#     The 0x1C filler makes the plane1 shift land exactly on 0x38 (+1.0)
#     with a single 2-op pass; bit0 cross-byte spill is masked by 0xB8B8.
#   - out.T[n, m] = sum_k wbT[k, n] * xbT[k, m] accumulates in PSUM with
#     DoubleRow fp8 matmuls (2 MACs/cell/cycle, contraction 256 per MM).
#   - ScalarE evacuates PSUM with a fused per-partition bias add
#     (activation Identity, bias = bias[n] column) directly to f16 output
#     tiles (values are +-2048-range integers + bias: f16 keeps rel err
#     ~2e-4, and halves output HBM traffic 16MB -> 8MB).
#   - Output DMAs ride the GpSimd queue (idle engine) so they never delay
#     PSUM evacuation on ScalarE or the input streams on Sync/ScalarE
#     queues.
#   - PE warm-up: 14 dummy DoubleRow matmuls on a garbage tile flip the HAM
#     clock gate (needs ~3.4us of sustained PE activity at the boot-time
#     half clock) and bridge gap-free into the first real matmul at
#     strip0-ready (~13us).  They accumulate into the PSUM pair that the
#     first real group touches LAST (ps1_1), so no real matmul ever waits
#     on the warm-up chain.  Trailing dummy matmuls after the last real one
#     keep the clock at full speed through the output drain.
#   - Host transposes each core's out.T shard back, casts f32, and stacks.

import sys

import numpy as np

try:
    import concourse  # noqa: F401
except ImportError:
    sys.path.insert(0, "/opt/trn_rl_repo")

from contextlib import ExitStack

import concourse.bass as bass
import concourse.mybir as mybir
import concourse.tile as tile
from concourse import bacc
from concourse.bass_utils import run_bass_kernel_spmd

NCORES = 8
K = 2048          # contraction dim (in_features)
NF = 2048         # out features
MTOT = 16384      # rows of x
MS = MTOT // NCORES  # rows per core
P = 128           # partitions
MC = 512          # moving free-dim chunk (one PSUM bank of f32)
KT2 = K // (2 * P)   # 8 double-k-tiles (DoubleRow contracts 256/MM)
NT = NF // P      # 16 n-tiles
MT = MS // MC     # 4 m-chunks

F32 = mybir.dt.float32
F16 = mybir.dt.float16
FP8 = mybir.dt.float8e4
U8 = mybir.dt.uint8
U16 = mybir.dt.uint16


def build_nc(debug=False):
    nc = bacc.Bacc(
        "TRN2", target_bir_lowering=False, debug=debug, num_devices=NCORES
    )
    # Sign-packed inputs: [t, p, c] u8, one contiguous 256KB DMA per strip
    # (2KB per partition line).
    xT = nc.dram_tensor("xT", [KT2, P, MS], U8, kind="ExternalInput").ap()
    wT = nc.dram_tensor("wT", [KT2, P, NF], U8, kind="ExternalInput").ap()
    # bias arrives pre-tiled [128, 16] from the host (column t holds
    # bias[t*128:(t+1)*128]) so the DMA is one contiguous 8KB transfer.
    bias = nc.dram_tensor("bias", [P, NT], F32, kind="ExternalInput").ap()
    outT = nc.dram_tensor("outT", [NF, MS], F16, kind="ExternalOutput").ap()

    NG = 2  # n-tiles per group; NG*MT psum banks live at once

    with tile.TileContext(nc) as tc:
        with ExitStack() as ctx:
            const = ctx.enter_context(tc.tile_pool(name="const", bufs=1))
            res = ctx.enter_context(tc.tile_pool(name="res", bufs=1))
            psum = ctx.enter_context(
                tc.tile_pool(name="ps", bufs=1, space=bass.MemorySpace.PSUM)
            )
            outp = ctx.enter_context(tc.tile_pool(name="out", bufs=3))

            # PE warm-up (see header).  Memset is Vector's first op (Vector
            # is idle until the first strip lands ~4us later), so the
            # warm-up starts right at the PE preamble end (~6.5us), flips
            # the HAM clock at ~10us, and bridges gap-free into the first
            # real matmul at strip0-ready (~11.5us): the clock gate needs
            # ~3.4us of CONTINUOUS PE activity, so a gap between warm-up
            # and first real MM would push the flip ~4us later.
            warm = const.tile([P, 2, MC], FP8, name="warm")
            nc.vector.memset(warm[:], 0.0)
            warm_ps = psum.tile([P, 2 * MC], F32, tag="ps1_1", name="warm_ps")

            def load_bin(dram, name, t, dma_eng, alu_eng, nsplit=1):
                # Packed strip [128, 2048] u8 DMAs into the j=0 plane of the
                # [128, 2, NF] fp8 operand tile; two tensor_scalar passes
                # expand to exact +-1.0 (plane1 first: it reads raw packed
                # bytes that the plane0 pass overwrites in place).  nsplit>1
                # loads/expands the strip in column slabs: Tile dependencies
                # are byte-range-based, so the first matmuls fire as soon as
                # their slab is ready -- used on strip 0 to get real work to
                # the PE right at the HAM clock flip instead of waiting for
                # the whole strip.
                tl = res.tile([P, 2, NF], FP8, tag=f"{name}{t}")
                cs = NF // nsplit
                for s in range(nsplit):
                    sl = slice(s * cs, (s + 1) * cs)
                    dma_eng.dma_start(
                        out=tl[:, 0, sl].bitcast(U8), in_=dram[t][:, sl]
                    )
                    alu_eng.tensor_scalar(
                        tl[:, 1, sl].bitcast(U16),
                        tl[:, 0, sl].bitcast(U16),
                        1,
                        0xB8B8,
                        mybir.AluOpType.logical_shift_left,
                        mybir.AluOpType.bitwise_and,
                    )
                    alu_eng.tensor_scalar(
                        tl[:, 0, sl].bitcast(U16),
                        tl[:, 0, sl].bitcast(U16),
                        0x8080,
                        0x3838,
                        mybir.AluOpType.bitwise_and,
                        mybir.AluOpType.bitwise_or,
                    )
                return tl

            # w strips on the sync queue, x strips on the scalar queue ->
            # the two HWDGE queues stream in parallel.  Both expand on
            # Vector (the Pool engine can't run tensor_scalar): 4 passes
            # of ~0.4us per strip-pair still beats the ~1.4us DMA cadence.
            # PE warm-up bridges the PE-preamble end (~6.5us) to strip-0
            # readiness (~13us) with CONTINUOUS activity: the HAM clock-flip
            # window resets on any PE gap, and a reset costs ~2us of
            # half-clock real matmuls.  14 warm-ups (8 at the 427ns half
            # clock + 6 at 216ns) measured as a seamless hand-off.  Slab-
            # splitting strip 0 to start real MMs earlier was tried twice
            # and regressed both times: Tile's cross-engine waits compile to
            # counting semaphores on the producer's instruction ordinal, so
            # extra early vector passes coarsen the first MM's dependency,
            # and the leftover warm-up gap delays the clock flip by ~4us.
            NWARM = 14
            for wi in range(NWARM):
                nc.tensor.matmul(
                    warm_ps[:, :MC],
                    warm[:, :, :P],
                    warm[:],
                    start=(wi == 0),
                    stop=(wi == NWARM - 1),
                    perf_mode=mybir.MatmulPerfMode.DoubleRow,
                )

            wb = []
            xb = []
            for t in range(KT2):
                wb.append(load_bin(wT, "w", t, nc.sync, nc.vector))
                xb.append(load_bin(xT, "x", t, nc.scalar, nc.vector))

            # bias is only needed at the first evacuation (~20us in); pushing
            # it after the strips keeps the w0 descriptor at the head of the
            # sync queue so strip 0 lands ~0.7us earlier.
            bias_t = const.tile([P, NT], F32)
            nc.sync.dma_start(out=bias_t[:], in_=bias[:])

            def w_slice(t, n):
                return wb[t][:, :, n * P : (n + 1) * P]

            def x_slice(t, mc):
                return xb[t][:, :, mc * MC : (mc + 1) * MC]

            # PSUM is organized as pair-tiles [128, 2*MC] spanning two banks:
            # each matmul still writes within a single bank (one MC slice),
            # but evacuation reads a whole pair in one ACTIVATE -- halving
            # the evacuation ops and the cross-engine semaphore edges (the
            # kernel epilogue's final drain pays ~tens of ns per allocated
            # semaphore, so edge count shows up on the wall clock).
            NPAIR = MT // 2
            NGRP = NT // NG

            def mm(ps_pair, g, i, mc, t):
                nc.tensor.matmul(
                    ps_pair[:, (mc % 2) * MC : (mc % 2 + 1) * MC],
                    w_slice(t, g * NG + i),
                    x_slice(t, mc),
                    start=(t == 0),
                    stop=(t == KT2 - 1),
                    perf_mode=mybir.MatmulPerfMode.DoubleRow,
                )

            for g in range(NGRP):
                pss = [
                    [
                        psum.tile(
                            [P, 2 * MC], F32, tag=f"ps{i}_{pr}", name=f"ps_{g}_{i}_{pr}"
                        )
                        for pr in range(NPAIR)
                    ]
                    for i in range(NG)
                ]
                ots = [
                    outp.tile([P, MS], F16, tag=f"o{i}", name=f"o_{g}_{i}")
                    for i in range(NG)
                ]

                def evacuate(i, pr, dma):
                    n = g * NG + i
                    if dma == "chunk":
                        # Final pair: evacuate + DMA in two 512-col chunks on
                        # the Sync queue (idle once inputs land, and not
                        # backed up behind the earlier output transfers like
                        # GpSimd's), so the tail is one small chunk's
                        # epilogue.
                        for c in range(2):
                            sl = slice(
                                (2 * pr + c) * MC, (2 * pr + c + 1) * MC
                            )
                            nc.scalar.activation(
                                ots[i][:, sl],
                                pss[i][pr][:, c * MC : (c + 1) * MC],
                                mybir.ActivationFunctionType.Identity,
                                bias=bias_t[:, n : n + 1],
                            )
                            nc.sync.dma_start(
                                out=outT[n * P : (n + 1) * P, sl],
                                in_=ots[i][:, sl],
                            )
                        return
                    nc.scalar.activation(
                        ots[i][:, pr * 2 * MC : (pr + 1) * 2 * MC],
                        pss[i][pr][:],
                        mybir.ActivationFunctionType.Identity,
                        bias=bias_t[:, n : n + 1],
                    )
                    # Output DMAs ride the (otherwise idle) GpSimd queue.
                    # dma=None batches the whole n-tile into one transfer;
                    # the last group DMAs per-pair for tail overlap.
                    if dma == "pair":
                        nc.sync.dma_start(
                            out=outT[
                                n * P : (n + 1) * P, pr * 2 * MC : (pr + 1) * 2 * MC
                            ],
                            in_=ots[i][:, pr * 2 * MC : (pr + 1) * 2 * MC],
                        )
                    elif dma == "tile":
                        nc.gpsimd.dma_start(
                            out=outT[n * P : (n + 1) * P, :], in_=ots[i][:]
                        )

                if g < NGRP - 1:
                    # k-tile outer: consume input strips as they stream in.
                    for t in range(KT2):
                        for i in range(NG):
                            for mc in range(MT):
                                mm(pss[i][mc // 2], g, i, mc, t)
                    for i in range(NG):
                        for pr in range(NPAIR):
                            evacuate(i, pr, "tile" if pr == NPAIR - 1 else None)
                else:
                    # Last group: pair-major so evacuation and output DMA of
                    # pair p overlap the matmuls of pair p+1 (shrinks the
                    # kernel tail to one pair's epilogue).
                    for i in range(NG):
                        for pr in range(NPAIR):
                            for mc in (2 * pr, 2 * pr + 1):
                                for t in range(KT2):
                                    mm(pss[i][pr], g, i, mc, t)
                            last = i == NG - 1 and pr == NPAIR - 1
                            evacuate(i, pr, "chunk" if last else "pair")

            # Trailing dummy matmuls: the HAM clock gate halves the clock
            # ~3.3us after the PE goes idle, which would put the final output
            # drain and the fixed ~250-semaphore epilogue wipe at half speed.
            # ~16 throwaway DoubleRow MMs (3.5us) keep the PE "busy" through
            # the drain window so the teardown runs at full clock.  They
            # reuse the ps0_0 bank (evacuated early in the last group) and
            # are never read.
            tail_ps = psum.tile([P, 2 * MC], F32, tag="ps0_0", name="tail_ps")
            NDUMMY = 16
            for wi in range(NDUMMY):
                nc.tensor.matmul(
                    tail_ps[:, :MC],
                    warm[:, :, :P],
                    warm[:],
                    start=(wi == 0),
                    stop=(wi == NDUMMY - 1),
                    perf_mode=mybir.MatmulPerfMode.DoubleRow,
                )

    nc.compile()
    return nc


_NC = None


def _get_nc():
    global _NC
    if _NC is None:
        _NC = build_nc()
    return _NC


def _pack_T(a):
    # Transposed sign-packed copy: byte = s_j0<<7 | s_j1<<6 | 0x1C with
    # s = (elem <= 0)  (reference binarize maps 0 -> -1).  j indexes the two
    # DoubleRow k-planes: element [t, j, p, c] = a.T[(2t+j)*128 + p, c].
    at = a.T
    kk, cols = at.shape
    s = (at <= 0).reshape(kk // (2 * P), 2, P, cols)
    pk = (
        (s[:, 0].astype(np.uint8) << 7)
        | (s[:, 1].astype(np.uint8) << 6)
        | np.uint8(0x1C)
    )
    return np.ascontiguousarray(pk)


def make_in_maps(x, weight, bias):
    x = np.asarray(x, dtype=np.float32)
    weight = np.asarray(weight, dtype=np.float32)
    bias = np.asarray(bias, dtype=np.float32)
    wTb = _pack_T(weight)
    bias_tiled = np.ascontiguousarray(bias.reshape(NT, P).T)
    in_maps = []
    for i in range(NCORES):
        xTb = _pack_T(x[i * MS : (i + 1) * MS, :])
        in_maps.append({"xT": xTb, "wT": wTb, "bias": bias_tiled})
    return in_maps


def assemble_out(results):
    out = np.empty((MTOT, NF), dtype=np.float32)
    for i in range(NCORES):
        out[i * MS : (i + 1) * MS, :] = results[i]["outT"].T.astype(np.float32)
    return out


def run(x, weight, bias, trace=False, **kwargs):
    nc = _get_nc()
    in_maps = make_in_maps(x, weight, bias)
    res = run_bass_kernel_spmd(
        nc, in_maps, list(range(NCORES)), trace=trace, **kwargs
    )
    return assemble_out(res.results), res


def kernel(x, weight, bias):
    out, _ = run(x, weight, bias)
    return out
